# revision 15
# baseline (speedup 1.0000x reference)
"""Trainium2 Bass kernel for nn_Decoder (embedding_lookup decoder).

reference:
    h = relu(latent @ W1 + b1); h = bn affine (eval)          [B, H]
    lw = logit_weight[genes_oi]                               [G, H, K]
    rw = rho_weight[genes_oi]                                 [G, H]
    logit = einsum("bh,ghk->bgk", h, lw)                      [B, G, K]
    rho   = einsum("bh,gh->bg", h, rw)                        [B, G]

Sharding: genes_oi split across 8 NeuronCores along the gene axis
(625 genes/core, padded to 640 = 5 tiles of 128); latent batch and the
tiny MLP replicated; the full weight tables replicated in each core's
DRAM and gathered on-device by indirect DMA.

Per-core layout trick: the host packs a combined per-gene row
    row[n] = concat(logit_weight[n].T.flatten(), rho_weight[n])  # 2080 f32
so a 128-gene gather lands one gene per SBUF partition with free index
f = 32*k + h.  A DVE 32x32 block transpose then yields
    T[32a + h, 32k + j] = logit_weight[gene(a,j), h, k]
i.e. contraction dim h on partitions for each 32-gene group a, which
feeds row-tiled fp32 matmuls (tile_position=(32a,0)) with the
replicated hT as the stationary operand.  PSUM results are copied
(ScalarE/VectorE) into [128, 8192] staging tiles and stored with 4 MB
DMAs -- the kernel is output-bandwidth bound (~82 MB/core).
"""

import sys

sys.path.insert(0, "/opt/trn_rl_repo")

import numpy as np

import bass_rust
from bass_rust import ScopedClock

import concourse.bass as bass
import concourse.mybir as mybir
import concourse.tile as tile
from concourse.bass_utils import run_bass_kernel_spmd

F32 = mybir.dt.float32
I32 = mybir.dt.int32

B = 512          # latent batch
NL = 64          # n_latent
H = 32           # hidden
KD = 64          # n_components (k)
NG = 20000       # n_genes in table
G = 5000         # genes_oi
NCORES = 8
GC = G // NCORES         # genes per core (625)
GP = 640                 # padded genes per core
T = GP // 128            # gene tiles per core (5)
NB = B // 128            # batch chunks (4)
ROW = (KD + 1) * H       # packed table row: 2048 logit-T + 32 rho
BN_EPS = 1e-5


def _split_multi_waits(nc):
    """The walrus build in this container rejects more than one sync-wait
    per instruction ("Too many sync wait commands").  Tile's semaphore
    assignment freely attaches several waits to one instruction, so after
    tracing, hoist all-but-the-last wait of every instruction into
    standalone single-wait EventSemaphore instructions on the same engine
    immediately before it (same-engine program order preserves the
    wait-then-execute semantics)."""
    fn = nc.m.functions[0]
    n = 0
    for bb in fn.blocks:
        new = []
        for inst in bb.instructions:
            si = inst.sync_info
            waits = list(si.on_wait) if si is not None else []
            if len(waits) > 1:
                imm = [w for w in waits if w.wait_mode == "sem-ge-imm"]
                rest = [w for w in waits if w.wait_mode != "sem-ge-imm"]
                assert len(rest) <= 1, (inst.name, waits)
                keep = rest if rest else imm[-1:]
                hoist = imm if rest else imm[:-1]
                for w in hoist:
                    n += 1
                    pre = mybir.InstNoOp(
                        name=f"{inst.name}_sw{n}",
                        engine=inst.engine,
                        bass_nofuse=True,
                        ins=[],
                        outs=[],
                        sync_info=bass_rust.SyncInfo(on_wait=[w], on_update=[]),
                    )
                    nc.register_instruction(pre)
                    new.append(pre)
                inst.sync_info = bass_rust.SyncInfo(
                    on_wait=keep, on_update=list(si.on_update)
                )
            new.append(inst)
        bb.instructions = new


def _build_program():
    nc = bass.Bass()

    lwt = nc.dram_tensor("lwt", [NG, ROW], F32, kind="ExternalInput")
    latT = nc.dram_tensor("latT", [NL, B], F32, kind="ExternalInput")
    w1r = nc.dram_tensor("w1r", [NL, 128], F32, kind="ExternalInput")
    b1r = nc.dram_tensor("b1r", [128], F32, kind="ExternalInput")
    gmr = nc.dram_tensor("gmr", [128], F32, kind="ExternalInput")
    btr = nc.dram_tensor("btr", [128], F32, kind="ExternalInput")
    genes = nc.dram_tensor("genes", [GP], I32, kind="ExternalInput")

    logit_out = nc.dram_tensor("logit_out", [B, GP, KD], F32, kind="ExternalOutput")
    rho_out = nc.dram_tensor("rho_out", [B, GP], F32, kind="ExternalOutput")

    with tile.TileContext(nc) as tc:
        with (
            tc.tile_pool(name="const", bufs=1) as cpool,
            tc.tile_pool(name="gather", bufs=2) as gpool,
            tc.tile_pool(name="tpose", bufs=2) as tpool,
            tc.tile_pool(name="stage", bufs=3) as spool,
            tc.tile_pool(name="psum", bufs=1, space="PSUM") as ppool,
        ):
            # ---- constant loads ----
            latT_sb = cpool.tile([NL, B], F32)
            nc.sync.dma_start(out=latT_sb[:], in_=latT[:])
            w1_sb = cpool.tile([NL, 128], F32)
            nc.sync.dma_start(out=w1_sb[:], in_=w1r[:])
            b1_sb = cpool.tile([128, 1], F32)
            nc.sync.dma_start(out=b1_sb[:], in_=b1r[:, None])
            gm_sb = cpool.tile([128, 1], F32)
            nc.sync.dma_start(out=gm_sb[:], in_=gmr[:, None])
            bt_sb = cpool.tile([128, 1], F32)
            nc.sync.dma_start(out=bt_sb[:], in_=btr[:, None])
            genes_sb = cpool.tile([128, T], I32)
            nc.sync.dma_start(
                out=genes_sb[:], in_=genes.rearrange("(t p) -> p t", p=128)
            )

            # ---- replicated MLP: hT[32a+h, b] = h_bn[b, h] ----
            hpre = ppool.tile([128, B], F32, tag="rho", bufs=4)
            nc.tensor.matmul(
                hpre[:], lhsT=w1_sb[:], rhs=latT_sb[:], start=True, stop=True
            )
            hrelu = cpool.tile([128, B], F32)
            nc.scalar.activation(
                hrelu[:], hpre[:], mybir.ActivationFunctionType.Relu,
                bias=b1_sb[:, :1], scale=1.0,
            )
            hT = cpool.tile([128, B], F32)
            nc.vector.tensor_scalar(
                out=hT[:], in0=hrelu[:], scalar1=gm_sb[:, :1], scalar2=bt_sb[:, :1],
                op0=mybir.AluOpType.mult, op1=mybir.AluOpType.add,
            )

            rho_stage = cpool.tile([128, NB * GP], F32)

            for t in range(T):
                gt = gpool.tile([128, ROW], F32)
                nc.gpsimd.indirect_dma_start(
                    out=gt[:],
                    out_offset=None,
                    in_=lwt[:],
                    in_offset=bass.IndirectOffsetOnAxis(
                        ap=genes_sb[:, t : t + 1], axis=0
                    ),
                )
                tt = tpool.tile([128, ROW], F32)
                nc.vector.transpose(tt[:], gt[:])
                # free-dim view: f = 32*k + j  ->  [h, j, k]
                tt_gk = [
                    tt[32 * a : 32 * a + 32, 0 : KD * H].rearrange(
                        "h (k g) -> h g k", k=KD, g=32
                    )
                    for a in range(4)
                ]

                # ---- rho: rho[b, gene(a,j)] = sum_h hT[32a+h, b] * tt[32a+h, 2048+j]
                # one PSUM bank per row group -- concurrent row-tiled matmuls
                # must not share a PSUM bank (hardware fault observed).
                for c in range(NB):
                    for a in range(4):
                        rho_ps = ppool.tile([128, 32], F32, tag="rho", bufs=4)
                        nc.tensor.matmul(
                            rho_ps[:],
                            lhsT=hT[32 * a : 32 * a + 32, 128 * c : 128 * c + 128],
                            rhs=tt[32 * a : 32 * a + 32, KD * H : KD * H + 32],
                            start=True, stop=True,
                            tile_position=(32 * a, 0),
                        )
                        nc.vector.tensor_copy(
                            rho_stage[
                                :,
                                GP * c + 128 * t + 32 * a : GP * c + 128 * t + 32 * a + 32,
                            ],
                            rho_ps[:],
                        )

                # ---- logit ----
                # the last tile holds genes 512..625: its (a=3, q=3) chunk
                # (genes 632..640) is pure padding -- skip compute and trim
                # the store to 113 genes.
                g_valid = min(128, GC - 128 * t)
                for c in range(NB):
                    st = spool.tile([128, 128 * KD], F32)
                    ncopy = 0
                    for q in range(4):
                        for a in range(4):
                            if 32 * a + 8 * q >= g_valid:
                                continue
                            ps = ppool.tile([128, 512], F32, tag="big", bufs=4)
                            nc.tensor.matmul(
                                ps[:],
                                lhsT=hT[
                                    32 * a : 32 * a + 32, 128 * c : 128 * c + 128
                                ],
                                rhs=tt_gk[a][:, 8 * q : 8 * q + 8, :],
                                start=True, stop=True,
                                tile_position=(32 * a, 0),
                            )
                            dst = st[:, 2048 * a + 512 * q : 2048 * a + 512 * q + 512]
                            # split PSUM->SBUF copies across ScalarE and VectorE
                            if ncopy % 16 < 9:
                                nc.scalar.copy(dst, ps[:])
                            else:
                                nc.vector.tensor_copy(dst, ps[:])
                            ncopy += 1
                    nc.sync.dma_start(
                        out=logit_out[
                            128 * c : 128 * c + 128,
                            128 * t : 128 * t + g_valid,
                            :,
                        ],
                        in_=st[:, : g_valid * KD],
                    )

            for c in range(NB):
                nc.sync.dma_start(
                    out=rho_out[128 * c : 128 * c + 128, :GC],
                    in_=rho_stage[:, GP * c : GP * c + GC],
                )

    _split_multi_waits(nc)
    return nc


_PROGRAM = None
LAST_RESULTS = None


def kernel(**inputs):
    global _PROGRAM, LAST_RESULTS

    latent = np.asarray(inputs["latent"], dtype=np.float32)          # [B, NL]
    genes_oi = np.asarray(inputs["genes_oi"]).astype(np.int32)       # [G]
    W1 = np.asarray(inputs["W1"], dtype=np.float32)                  # [NL, H]
    b1 = np.asarray(inputs["b1"], dtype=np.float32)                  # [H]
    bn_gamma = np.asarray(inputs["bn_gamma"], dtype=np.float32)
    bn_beta = np.asarray(inputs["bn_beta"], dtype=np.float32)
    bn_mean = np.asarray(inputs["bn_mean"], dtype=np.float32)
    bn_var = np.asarray(inputs["bn_var"], dtype=np.float32)
    logit_weight = np.asarray(inputs["logit_weight"], dtype=np.float32)  # [NG,H,KD]
    rho_weight = np.asarray(inputs["rho_weight"], dtype=np.float32)      # [NG,H]

    # packed per-gene row: [logit_weight[n].T (k-major), rho_weight[n]]
    lwt = np.empty((NG, ROW), dtype=np.float32)
    lwt[:, : KD * H] = logit_weight.transpose(0, 2, 1).reshape(NG, KD * H)
    lwt[:, KD * H :] = rho_weight

    latT = np.ascontiguousarray(latent.T)                    # [NL, B]
    w1r = np.tile(W1, (1, 4))                                # [NL, 128]
    gamma_c = bn_gamma / np.sqrt(bn_var + BN_EPS)
    beta_c = bn_beta - gamma_c * bn_mean
    b1r = np.tile(b1, 4)
    gmr = np.tile(gamma_c, 4)
    btr = np.tile(beta_c, 4)

    if _PROGRAM is None:
        _PROGRAM = _build_program()

    in_maps = []
    for i in range(NCORES):
        g = np.zeros(GP, dtype=np.int32)
        g[:GC] = genes_oi[i * GC : (i + 1) * GC]
        in_maps.append(
            {
                "lwt": lwt,
                "latT": latT,
                "w1r": w1r,
                "b1r": b1r,
                "gmr": gmr,
                "btr": btr,
                "genes": g,
            }
        )

    res = run_bass_kernel_spmd(_PROGRAM, in_maps, core_ids=list(range(NCORES)))
    LAST_RESULTS = res

    logit = np.concatenate(
        [res.results[i]["logit_out"][:, :GC, :] for i in range(NCORES)], axis=1
    )
    rho = np.concatenate(
        [res.results[i]["rho_out"][:, :GC] for i in range(NCORES)], axis=1
    )
    return logit, rho


# revision 16
# speedup vs baseline: 1.0213x; 1.0213x over previous
"""Trainium2 Bass kernel for nn_Decoder (embedding_lookup decoder).

reference:
    h = relu(latent @ W1 + b1); h = bn affine (eval)          [B, H]
    lw = logit_weight[genes_oi]                               [G, H, K]
    rw = rho_weight[genes_oi]                                 [G, H]
    logit = einsum("bh,ghk->bgk", h, lw)                      [B, G, K]
    rho   = einsum("bh,gh->bg", h, rw)                        [B, G]

Sharding: genes_oi split across 8 NeuronCores along the gene axis
(625 genes/core, padded to 640 = 5 tiles of 128); latent batch and the
tiny MLP replicated; the full weight tables replicated in each core's
DRAM and gathered on-device by indirect DMA.

Per-core layout trick: the host packs a combined per-gene row
    row[n] = concat(logit_weight[n].T.flatten(), rho_weight[n])  # 2080 f32
so a 128-gene gather lands one gene per SBUF partition with free index
f = 32*k + h.  A DVE 32x32 block transpose then yields
    T[32a + h, 32k + j] = logit_weight[gene(a,j), h, k]
i.e. contraction dim h on partitions for each 32-gene group a, which
feeds row-tiled fp32 matmuls (tile_position=(32a,0)) with the
replicated hT as the stationary operand.  PSUM results are copied
(ScalarE/VectorE) into [128, 8192] staging tiles and stored with 4 MB
DMAs -- the kernel is output-bandwidth bound (~82 MB/core).
"""

import sys

sys.path.insert(0, "/opt/trn_rl_repo")

import numpy as np

import bass_rust
from bass_rust import ScopedClock

import concourse.bass as bass
import concourse.mybir as mybir
import concourse.tile as tile
from concourse.bass_utils import run_bass_kernel_spmd

F32 = mybir.dt.float32
I32 = mybir.dt.int32

B = 512          # latent batch
NL = 64          # n_latent
H = 32           # hidden
KD = 64          # n_components (k)
NG = 20000       # n_genes in table
G = 5000         # genes_oi
NCORES = 8
GC = G // NCORES         # genes per core (625)
GP = 640                 # padded genes per core
T = GP // 128            # gene tiles per core (5)
NB = B // 128            # batch chunks (4)
ROW = (KD + 1) * H       # packed table row: 2048 logit-T + 32 rho
BN_EPS = 1e-5


def _split_multi_waits(nc):
    """The walrus build in this container rejects more than one sync-wait
    per instruction ("Too many sync wait commands").  Tile's semaphore
    assignment freely attaches several waits to one instruction, so after
    tracing, hoist all-but-the-last wait of every instruction into
    standalone single-wait EventSemaphore instructions on the same engine
    immediately before it (same-engine program order preserves the
    wait-then-execute semantics)."""
    fn = nc.m.functions[0]
    n = 0
    for bb in fn.blocks:
        new = []
        for inst in bb.instructions:
            si = inst.sync_info
            waits = list(si.on_wait) if si is not None else []
            if len(waits) > 1:
                imm = [w for w in waits if w.wait_mode == "sem-ge-imm"]
                rest = [w for w in waits if w.wait_mode != "sem-ge-imm"]
                assert len(rest) <= 1, (inst.name, waits)
                keep = rest if rest else imm[-1:]
                hoist = imm if rest else imm[:-1]
                for w in hoist:
                    n += 1
                    pre = mybir.InstNoOp(
                        name=f"{inst.name}_sw{n}",
                        engine=inst.engine,
                        bass_nofuse=True,
                        ins=[],
                        outs=[],
                        sync_info=bass_rust.SyncInfo(on_wait=[w], on_update=[]),
                    )
                    nc.register_instruction(pre)
                    new.append(pre)
                inst.sync_info = bass_rust.SyncInfo(
                    on_wait=keep, on_update=list(si.on_update)
                )
            new.append(inst)
        bb.instructions = new


def _build_program():
    nc = bass.Bass()

    lwt = nc.dram_tensor("lwt", [NG, ROW], F32, kind="ExternalInput")
    latT = nc.dram_tensor("latT", [NL, B], F32, kind="ExternalInput")
    w1r = nc.dram_tensor("w1r", [NL, 128], F32, kind="ExternalInput")
    b1r = nc.dram_tensor("b1r", [128], F32, kind="ExternalInput")
    gmr = nc.dram_tensor("gmr", [128], F32, kind="ExternalInput")
    btr = nc.dram_tensor("btr", [128], F32, kind="ExternalInput")
    genes = nc.dram_tensor("genes", [GP], I32, kind="ExternalInput")

    logit_out = nc.dram_tensor("logit_out", [B, GP, KD], F32, kind="ExternalOutput")
    rho_out = nc.dram_tensor("rho_out", [B, GP], F32, kind="ExternalOutput")

    with tile.TileContext(nc) as tc:
        with (
            tc.tile_pool(name="const", bufs=1) as cpool,
            tc.tile_pool(name="gather", bufs=2) as gpool,
            tc.tile_pool(name="tpose", bufs=2) as tpool,
            tc.tile_pool(name="stage", bufs=3) as spool,
            tc.tile_pool(name="psum", bufs=1, space="PSUM") as ppool,
        ):
            # ---- constant loads ----
            # gene indices go first and via GpSimd: the indirect gathers
            # (also GpSimd/SWDGE) depend only on them, so the first gather
            # can start while the sync queue streams the other constants.
            genes_sb = cpool.tile([128, T], I32)
            nc.gpsimd.dma_start(
                out=genes_sb[:], in_=genes.rearrange("(t p) -> p t", p=128)
            )
            latT_sb = cpool.tile([NL, B], F32)
            nc.sync.dma_start(out=latT_sb[:], in_=latT[:])
            w1_sb = cpool.tile([NL, 128], F32)
            nc.sync.dma_start(out=w1_sb[:], in_=w1r[:])
            b1_sb = cpool.tile([128, 1], F32)
            nc.sync.dma_start(out=b1_sb[:], in_=b1r[:, None])
            gm_sb = cpool.tile([128, 1], F32)
            nc.sync.dma_start(out=gm_sb[:], in_=gmr[:, None])
            bt_sb = cpool.tile([128, 1], F32)
            nc.sync.dma_start(out=bt_sb[:], in_=btr[:, None])
            # ---- replicated MLP: hT[32a+h, b] = h_bn[b, h] ----
            hpre = ppool.tile([128, B], F32, tag="rho", bufs=4)
            nc.tensor.matmul(
                hpre[:], lhsT=w1_sb[:], rhs=latT_sb[:], start=True, stop=True
            )
            hrelu = cpool.tile([128, B], F32)
            nc.scalar.activation(
                hrelu[:], hpre[:], mybir.ActivationFunctionType.Relu,
                bias=b1_sb[:, :1], scale=1.0,
            )
            hT = cpool.tile([128, B], F32)
            nc.vector.tensor_scalar(
                out=hT[:], in0=hrelu[:], scalar1=gm_sb[:, :1], scalar2=bt_sb[:, :1],
                op0=mybir.AluOpType.mult, op1=mybir.AluOpType.add,
            )

            rho_stage = cpool.tile([128, NB * GP], F32)

            for t in range(T):
                gt = gpool.tile([128, ROW], F32)
                nc.gpsimd.indirect_dma_start(
                    out=gt[:],
                    out_offset=None,
                    in_=lwt[:],
                    in_offset=bass.IndirectOffsetOnAxis(
                        ap=genes_sb[:, t : t + 1], axis=0
                    ),
                )
                tt = tpool.tile([128, ROW], F32)
                nc.vector.transpose(tt[:], gt[:])
                # free-dim view: f = 32*k + j  ->  [h, j, k]
                tt_gk = [
                    tt[32 * a : 32 * a + 32, 0 : KD * H].rearrange(
                        "h (k g) -> h g k", k=KD, g=32
                    )
                    for a in range(4)
                ]

                # ---- logit ----
                # the last tile holds genes 512..625: its (a=3, q=3) chunk
                # (genes 632..640) is pure padding -- skip compute and trim
                # the store to 113 genes.
                g_valid = min(128, GC - 128 * t)
                for c in range(NB):
                    st = spool.tile([128, 128 * KD], F32)
                    ncopy = 0
                    for q in range(4):
                        for a in range(4):
                            if 32 * a + 8 * q >= g_valid:
                                continue
                            ps = ppool.tile([128, 512], F32, tag="big", bufs=4)
                            nc.tensor.matmul(
                                ps[:],
                                lhsT=hT[
                                    32 * a : 32 * a + 32, 128 * c : 128 * c + 128
                                ],
                                rhs=tt_gk[a][:, 8 * q : 8 * q + 8, :],
                                start=True, stop=True,
                                tile_position=(32 * a, 0),
                            )
                            dst = st[:, 2048 * a + 512 * q : 2048 * a + 512 * q + 512]
                            # split PSUM->SBUF copies across ScalarE and VectorE
                            if ncopy % 16 < 9:
                                nc.scalar.copy(dst, ps[:])
                            else:
                                nc.vector.tensor_copy(dst, ps[:])
                            ncopy += 1
                    nc.sync.dma_start(
                        out=logit_out[
                            128 * c : 128 * c + 128,
                            128 * t : 128 * t + g_valid,
                            :,
                        ],
                        in_=st[:, : g_valid * KD],
                    )

                # ---- rho: rho[b, gene(a,j)] = sum_h hT[32a+h, b] * tt[32a+h, 2048+j]
                # one PSUM bank per row group -- concurrent row-tiled matmuls
                # must not share a PSUM bank (hardware fault observed).
                for c in range(NB):
                    for a in range(4):
                        rho_ps = ppool.tile([128, 32], F32, tag="rho", bufs=4)
                        nc.tensor.matmul(
                            rho_ps[:],
                            lhsT=hT[32 * a : 32 * a + 32, 128 * c : 128 * c + 128],
                            rhs=tt[32 * a : 32 * a + 32, KD * H : KD * H + 32],
                            start=True, stop=True,
                            tile_position=(32 * a, 0),
                        )
                        nc.vector.tensor_copy(
                            rho_stage[
                                :,
                                GP * c + 128 * t + 32 * a : GP * c + 128 * t + 32 * a + 32,
                            ],
                            rho_ps[:],
                        )


            for c in range(NB):
                nc.sync.dma_start(
                    out=rho_out[128 * c : 128 * c + 128, :GC],
                    in_=rho_stage[:, GP * c : GP * c + GC],
                )

    _split_multi_waits(nc)
    return nc


_PROGRAM = None
LAST_RESULTS = None


def kernel(**inputs):
    global _PROGRAM, LAST_RESULTS

    latent = np.asarray(inputs["latent"], dtype=np.float32)          # [B, NL]
    genes_oi = np.asarray(inputs["genes_oi"]).astype(np.int32)       # [G]
    W1 = np.asarray(inputs["W1"], dtype=np.float32)                  # [NL, H]
    b1 = np.asarray(inputs["b1"], dtype=np.float32)                  # [H]
    bn_gamma = np.asarray(inputs["bn_gamma"], dtype=np.float32)
    bn_beta = np.asarray(inputs["bn_beta"], dtype=np.float32)
    bn_mean = np.asarray(inputs["bn_mean"], dtype=np.float32)
    bn_var = np.asarray(inputs["bn_var"], dtype=np.float32)
    logit_weight = np.asarray(inputs["logit_weight"], dtype=np.float32)  # [NG,H,KD]
    rho_weight = np.asarray(inputs["rho_weight"], dtype=np.float32)      # [NG,H]

    # packed per-gene row: [logit_weight[n].T (k-major), rho_weight[n]]
    lwt = np.empty((NG, ROW), dtype=np.float32)
    lwt[:, : KD * H] = logit_weight.transpose(0, 2, 1).reshape(NG, KD * H)
    lwt[:, KD * H :] = rho_weight

    latT = np.ascontiguousarray(latent.T)                    # [NL, B]
    w1r = np.tile(W1, (1, 4))                                # [NL, 128]
    gamma_c = bn_gamma / np.sqrt(bn_var + BN_EPS)
    beta_c = bn_beta - gamma_c * bn_mean
    b1r = np.tile(b1, 4)
    gmr = np.tile(gamma_c, 4)
    btr = np.tile(beta_c, 4)

    if _PROGRAM is None:
        _PROGRAM = _build_program()

    in_maps = []
    for i in range(NCORES):
        g = np.zeros(GP, dtype=np.int32)
        g[:GC] = genes_oi[i * GC : (i + 1) * GC]
        in_maps.append(
            {
                "lwt": lwt,
                "latT": latT,
                "w1r": w1r,
                "b1r": b1r,
                "gmr": gmr,
                "btr": btr,
                "genes": g,
            }
        )

    res = run_bass_kernel_spmd(_PROGRAM, in_maps, core_ids=list(range(NCORES)))
    LAST_RESULTS = res

    logit = np.concatenate(
        [res.results[i]["logit_out"][:, :GC, :] for i in range(NCORES)], axis=1
    )
    rho = np.concatenate(
        [res.results[i]["rho_out"][:, :GC] for i in range(NCORES)], axis=1
    )
    return logit, rho
